# revision 1
# baseline (speedup 1.0000x reference)
"""Trainium2 Bass kernel for nn_Block_17738214932786 (spiking transformer block).

Computation (B=16, C=512, N=1024, H=8 heads, HID=2048):
    q = spike(bn(q_w @ x)); k,v likewise          (spikes are 0/1)
    attn = (Qh Kh^T) Vh * 0.25 == Qh (Kh^T Vh) * 0.25   (exact: integers)
    a = spike(attn)                               (threshold attn >= 8)
    a = spike(bn(proj_w @ a + proj_bias))
    x = x + a
    h = spike(bn(fc1_w @ x + fc1_bias))
    h = spike(bn(fc2_w @ h + fc2_bias))
    out = x + h

Strategy: data-parallel over batch across 8 NeuronCores (2 batches/core,
per-core activation matrix [512, 2048]). BatchNorm (training mode: stats
over batch*length) is handled sync-BN style: per-channel [mean, E[y^2]]
AllReduced per conv (tiny buffers); BN+LIF then collapses to a
per-channel threshold compare y >= t with
t = mean + (2 - bn_b) * sqrt(var + eps) / bn_g - conv_bias.

Precision: q/k/v/fc1 matmuls native fp32 (4-pass PE). Attention is exact
integer arithmetic in bf16: spikes are 0/1 and Kt@V integer counts
(<=1024) are split hi+lo bf16 losslessly, PSUM accumulates fp32.
proj/fc2 use hi+lo split bf16 weights (rhs is 0/1 so only the 2^-17
weight-split error remains).
"""

import sys
import types
import numpy as np

B, C, N, H = 16, 512, 1024, 8
D = C // H
HID = 4 * C
NCORES = 8
BPC = B // NCORES          # batches per core
COLS = BPC * N             # 2048
P = 128
NKC = C // P               # 4  tiles over C
NMH = HID // P             # 16 tiles over HID
NCH = COLS // 512          # 4  512-col chunks per core
NPT = COLS // P            # 16 col tiles per core
BN_EPS = 1e-5

_cache = {}


def _ensure_axon_hooks_shim():
    try:
        import antenv.axon_hooks  # noqa: F401
        return
    except Exception:
        pass
    m = types.ModuleType("antenv.axon_hooks")
    m.get_axon_ntff_profile_hook = lambda: None
    try:
        import antenv  # noqa: F401
    except Exception:
        sys.modules["antenv"] = types.ModuleType("antenv")
    sys.modules["antenv.axon_hooks"] = m


def _build_program():
    from contextlib import ExitStack
    import concourse.bacc as bacc
    import concourse.tile as tile
    from concourse import mybir
    from concourse.masks import make_identity

    dt = mybir.dt
    f32, bf16 = dt.float32, dt.bfloat16
    AF = mybir.ActivationFunctionType
    GE = mybir.AluOpType.is_ge
    RG = [list(range(NCORES))]

    nc = bacc.Bacc("TRN2", target_bir_lowering=False, debug=False,
                   num_devices=NCORES)

    x_in = nc.dram_tensor("x", [C, COLS], f32, kind="ExternalInput")
    wqkvT_in = nc.dram_tensor("wqkvT", [C, 3 * C], f32, kind="ExternalInput")
    wprojT_hi_in = nc.dram_tensor("wprojT_hi", [C, C], bf16, kind="ExternalInput")
    wprojT_lo_in = nc.dram_tensor("wprojT_lo", [C, C], bf16, kind="ExternalInput")
    wfc1T_hi_in = nc.dram_tensor("wfc1T_hi", [C, HID], bf16, kind="ExternalInput")
    wfc1T_lo_in = nc.dram_tensor("wfc1T_lo", [C, HID], bf16, kind="ExternalInput")
    wfc2T_hi_in = nc.dram_tensor("wfc2T_hi", [HID, C], bf16, kind="ExternalInput")
    wfc2T_lo_in = nc.dram_tensor("wfc2T_lo", [HID, C], bf16, kind="ExternalInput")
    thr_qkv_in = nc.dram_tensor("thr_qkv", [C, 6], f32, kind="ExternalInput")
    thr_proj_in = nc.dram_tensor("thr_proj", [C, 2], f32, kind="ExternalInput")
    thr_fc1_in = nc.dram_tensor("thr_fc1", [HID, 2], f32, kind="ExternalInput")
    thr_fc2_in = nc.dram_tensor("thr_fc2", [C, 2], f32, kind="ExternalInput")
    out_ext = nc.dram_tensor("out", [C, COLS], f32, kind="ExternalOutput")

    def part3(ap, p=P):  # [(m p), n] dram view -> [p, m, n]
        return ap.rearrange("(m p) n -> p m n", p=p)

    with tile.TileContext(nc, pool_alloc_mode="queue") as tc, ExitStack() as es:
        misc = es.enter_context(tc.tile_pool(name="misc", bufs=1))
        dram = es.enter_context(tc.tile_pool(name="dram", bufs=1, space="DRAM"))
        pp_mm = es.enter_context(tc.tile_pool(name="pp_mm", bufs=6, space="PSUM"))
        pp_sm = es.enter_context(tc.tile_pool(name="pp_sm", bufs=2, space="PSUM"))

        ident_bf = misc.tile([P, P], bf16)
        make_identity(nc, ident_bf)
        eps_t = misc.tile([P, 1], f32)
        nc.vector.memset(eps_t, BN_EPS)

        par_qkv = misc.tile([P, NKC, 6], f32)
        nc.gpsimd.dma_start(out=par_qkv, in_=part3(thr_qkv_in[:, :]))
        par_proj = misc.tile([P, NKC, 2], f32)
        nc.gpsimd.dma_start(out=par_proj, in_=part3(thr_proj_in[:, :]))
        par_fc1 = misc.tile([P, NMH, 2], f32)
        nc.gpsimd.dma_start(out=par_fc1, in_=part3(thr_fc1_in[:, :]))
        par_fc2 = misc.tile([P, NKC, 2], f32)
        nc.gpsimd.dma_start(out=par_fc2, in_=part3(thr_fc2_in[:, :]))

        # warmup collective: absorbs the first-call staging/skew cost so the
        # k-conv stats AllGather runs at steady-state latency
        dmy_in = dram.tile([P, 2], f32, name="dmy_in")
        dmy_out = dram.tile([NCORES, P, 2], f32, name="dmy_out")
        nc.gpsimd.collective_compute(
            "AllGather", mybir.AluOpType.bypass, replica_groups=RG,
            ins=[dmy_in.opt()], outs=[dmy_out.opt()])

        xres_d = dram.tile([NKC, P, COLS], f32, name="xres_dram")
        xrsh_d = dram.tile([NKC, P, COLS], bf16, name="xrsh_dram")
        xrsl_d = dram.tile([NKC, P, COLS], bf16, name="xrsl_dram")

        def stats_allreduce(name, pool, y_sb, nm):
            """Per-channel [mean_local, E2_local] -> AllReduce -> DRAM tile."""
            stats = pool.tile([P, nm, NCH, 6], f32, name=f"st_{name}")
            for m in range(nm):
                for n_ in range(NCH):
                    nc.vector.bn_stats(out=stats[:, m, n_, :],
                                       in_=y_sb[:, m, 512 * n_:512 * n_ + 512])
            return stats_finish(name, pool, stats, nm)

        def stats_finish(name, pool, stats, nm):
            mv = pool.tile([P, nm, 2], f32, name=f"mv_{name}")
            for m in range(nm):
                nc.vector.bn_aggr(out=mv[:, m, :], in_=stats[:, m, :, :])
            pack = pool.tile([P, nm, 2], f32, name=f"pk_{name}")
            nc.vector.tensor_mul(pack[:, :, 1], mv[:, :, 0], mv[:, :, 0])
            nc.vector.tensor_add(pack[:, :, 1], pack[:, :, 1], mv[:, :, 1])
            nc.vector.tensor_copy(pack[:, :, 0], mv[:, :, 0])
            bin_ = dram.tile([P, nm * 2], f32, name=f"arin_{name}")
            bout = dram.tile([NCORES, P, nm * 2], f32, name=f"arout_{name}")
            nc.sync.dma_start(out=bin_, in_=pack)
            nc.gpsimd.collective_compute(
                "AllGather", mybir.AluOpType.bypass, replica_groups=RG,
                ins=[bin_.opt()], outs=[bout.opt()])
            return bout

        def thresholds(name, pool, bout, thr_par, thr_col, nm):
            """AllGathered per-core stats -> reduce -> thresholds [P, nm]."""
            ag = pool.tile([P, NCORES, nm, 2], f32, name=f"ag_{name}")
            nc.sync.dma_start(out=ag, in_=bout.rearrange("r p c -> p r c"))
            # tree-reduce over the 8 ranks
            nc.vector.tensor_add(ag[:, 0:4], ag[:, 0:4], ag[:, 4:8])
            nc.vector.tensor_add(ag[:, 0:2], ag[:, 0:2], ag[:, 2:4])
            arst = pool.tile([P, nm, 2], f32, name=f"ar_{name}")
            nc.vector.tensor_add(arst, ag[:, 0, :, :], ag[:, 1, :, :])
            nc.vector.tensor_scalar_mul(arst, arst, 1.0 / NCORES)
            t_t = pool.tile([P, nm], f32, name=f"thr_{name}")
            tmp = pool.tile([P, nm], f32, name=f"tmp_{name}")
            nc.vector.tensor_mul(tmp, arst[:, :, 0], arst[:, :, 0])
            nc.vector.tensor_sub(tmp, arst[:, :, 1], tmp)
            nc.scalar.activation(out=tmp, in_=tmp, func=AF.Sqrt,
                                 bias=eps_t, scale=1.0)
            nc.vector.tensor_mul(tmp, tmp, thr_par[:, :, thr_col])
            nc.vector.tensor_add(t_t, tmp, arst[:, :, 0])
            nc.vector.tensor_sub(t_t, t_t, thr_par[:, :, thr_col + 1])
            return t_t

        # =============== Phases A+B+C under nested pools ===============
        def conv_mms(w_sb, rhs_tile, y_sb, stats, m_list, lhsT_col0=0):
            """fp32 conv matmuls for given m-tiles; fused psum->y copy and
            bn_stats (from PSUM)."""
            for m in m_list:
                for n_ in range(NCH):
                    ps = pp_mm.tile([P, 512], f32, name="ps_mm")
                    for k in range(NKC):
                        nc.tensor.matmul(
                            ps, lhsT=w_sb[:, k, lhsT_col0 + P * m:lhsT_col0 + P * m + P],
                            rhs=rhs_tile[:, k, 512 * n_:512 * n_ + 512],
                            start=(k == 0), stop=(k == NKC - 1))
                    nc.any.tensor_copy(y_sb[:, m, 512 * n_:512 * n_ + 512], ps)
                    nc.vector.bn_stats(out=stats[:, m, n_, :], in_=ps)

        def spike(dst, src, t_t, tcol):
            nc.vector.tensor_scalar(out=dst, in0=src,
                                    scalar1=t_t[:, tcol:tcol + 1],
                                    scalar2=None, op0=GE)

        def transposes(spkb, dstT):
            for m in range(NKC):
                for p_ in range(NPT):
                    pst = pp_sm.tile([P, P], bf16, name="ps_sm")
                    nc.tensor.transpose(pst, in_=spkb[:, m, P * p_:P * p_ + P],
                                        identity=ident_bf)
                    nc.any.tensor_copy(dstT[:, p_, P * m:P * m + P], pst)

        with tc.tile_pool(name="p_as", bufs=1) as p_as:   # a_spk: lives A..C
            a_spk = p_as.tile([P, NKC, COLS], bf16)

            with tc.tile_pool(name="p_ab", bufs=1) as p_ab:  # lives A..B
                q_spk = p_ab.tile([P, NKC, COLS], bf16)
                kT = p_ab.tile([P, NPT, C], bf16)
                vT = p_ab.tile([P, NPT, C], bf16)

                # ---- Phase A: k,v,q convs (fp32), interleaved so every
                # AllReduce hides under the next conv's matmuls ----
                with tc.tile_pool(name="p_ykv", bufs=2) as p_ykv:
                    with tc.tile_pool(name="p_xw", bufs=2) as p_xw:
                        def qkv_conv(ci, y_sb):
                            w_sb = p_xw.tile([P, NKC, C], f32, name="w_qkv", bufs=2)
                            nc.scalar.dma_start(
                                out=w_sb,
                                in_=part3(wqkvT_in[:, :])[:, :, 512 * ci:512 * ci + 512])
                            st = misc.tile([P, NKC, NCH, 6], f32, name=f"st_qkv{ci}")
                            for hf in range(2):
                                xc = p_xw.tile([P, NKC, 1024], f32, name="xc")
                                xsrc = part3(x_in[:, :])[:, :, 1024 * hf:1024 * hf + 1024]
                                nc.sync.dma_start(out=xc, in_=xsrc)
                                for m in range(NKC):
                                    pss = [pp_mm.tile([P, 512], f32, name="ps_mm")
                                           for _ in range(2)]
                                    for k in range(NKC):
                                        for n_ in range(2):
                                            nc.tensor.matmul(
                                                pss[n_], lhsT=w_sb[:, k, P * m:P * m + P],
                                                rhs=xc[:, k, 512 * n_:512 * n_ + 512],
                                                start=(k == 0), stop=(k == NKC - 1))
                                    for n_ in range(2):
                                        cc = 1024 * hf + 512 * n_
                                        nc.any.tensor_copy(
                                            y_sb[:, m, cc:cc + 512], pss[n_])
                                        nc.vector.bn_stats(
                                            out=st[:, m, 2 * hf + n_, :], in_=pss[n_])
                            return stats_finish(f"qkv{ci}", misc, st, NKC)

                        y_k = p_ykv.tile([P, NKC, COLS], f32, name="ybuf")
                        bout_k = qkv_conv(1, y_k)
                        y_v = p_ykv.tile([P, NKC, COLS], f32, name="ybuf")
                        bout_v = qkv_conv(2, y_v)
                        # k spikes (bf16) + transposes (AR-k already done)
                        t_k = thresholds("k", misc, bout_k, par_qkv, 2, NKC)
                        spkb_k = p_xw.tile([P, NKC, COLS], bf16, name="spkb",
                                           bufs=1)
                        for m in range(NKC):
                            spike(spkb_k[:, m, :], y_k[:, m, :], t_k, m)
                        transposes(spkb_k, kT)
                        y_q = p_ykv.tile([P, NKC, COLS], f32, name="ybuf")
                        bout_q = qkv_conv(0, y_q)
                        # v spikes + transposes (AR-v hidden under q conv)
                        t_v = thresholds("v", misc, bout_v, par_qkv, 4, NKC)
                        spkb_v = p_xw.tile([P, NKC, COLS], bf16, name="spkb",
                                           bufs=1)
                        for m in range(NKC):
                            spike(spkb_v[:, m, :], y_v[:, m, :], t_v, m)
                        transposes(spkb_v, vT)
                        # q spikes -> bf16 (DVE waits AR-q; PE continues with
                        # v-transposes and the kv matmuls of phase B)
                        t_q = thresholds("q", misc, bout_q, par_qkv, 0, NKC)
                        for m in range(NKC):
                            spike(q_spk[:, m, :], y_q[:, m, :], t_q, m)

                # ---- Phase B: attention (exact integer bf16) ----
                with tc.tile_pool(name="p_kv", bufs=4) as p_kv:
                    kvs = {}
                    for b in range(BPC):
                        for j in range(H // 2):   # head pairs -> blockdiag lhsT
                            blk_hi = p_kv.tile([P, P], bf16, name="kvblk_hi")
                            blk_lo = p_kv.tile([P, P], bf16, name="kvblk_lo")
                            nc.vector.memset(blk_hi, 0.0)
                            nc.vector.memset(blk_lo, 0.0)
                            pkv = pp_sm.tile([P, 64], f32, name="ps_sm")
                            for hh in range(2):
                                h_ = 2 * j + hh
                                sl = slice(64 * hh, 64 * hh + 64)
                                for t_ in range(N // P):
                                    nc.tensor.matmul(
                                        pkv[sl, :],
                                        lhsT=kT[:, (N // P) * b + t_, D * h_:D * h_ + D],
                                        rhs=vT[:, (N // P) * b + t_, D * h_:D * h_ + D],
                                        start=(t_ == 0), stop=(t_ == N // P - 1),
                                        tile_position=(0, 64 * hh))
                                # lossless integer split: hi=bf16(kv), lo=kv-hi
                                nc.any.tensor_copy(blk_hi[sl, sl], pkv[sl, :])
                                nc.vector.tensor_sub(blk_lo[sl, sl], pkv[sl, :],
                                                     blk_hi[sl, sl])
                            kvs[(b, j)] = (blk_hi, blk_lo)

                    for b in range(BPC):
                        for j in range(H // 2):
                            blk_hi, blk_lo = kvs[(b, j)]
                            pas = [pp_mm.tile([P, 512], f32, name="ps_mm")
                                   for _ in range(N // 512)]
                            for wi, blk in enumerate((blk_hi, blk_lo)):
                                for n_ in range(N // 512):
                                    cs = slice(N * b + 512 * n_, N * b + 512 * n_ + 512)
                                    nc.tensor.matmul(pas[n_], lhsT=blk,
                                                     rhs=q_spk[:, j, cs],
                                                     start=(wi == 0), stop=(wi == 1))
                            for n_ in range(N // 512):
                                cs = slice(N * b + 512 * n_, N * b + 512 * n_ + 512)
                                nc.vector.tensor_scalar(
                                    out=a_spk[:, j, cs], in0=pas[n_],
                                    scalar1=8.0, scalar2=None, op0=GE)

            # ---- Phase C: proj (split bf16) + residual -> xres_d ----
            with tc.tile_pool(name="p_pr", bufs=1) as p_pr, \
                 tc.tile_pool(name="p_prs", bufs=2) as p_prs:
                wpT_hi = p_pr.tile([P, NKC, C], bf16)
                nc.sync.dma_start(out=wpT_hi, in_=part3(wprojT_hi_in[:, :]))
                wpT_lo = p_pr.tile([P, NKC, C], bf16)
                nc.sync.dma_start(out=wpT_lo, in_=part3(wprojT_lo_in[:, :]))

                y_p = p_pr.tile([P, NKC, COLS], f32)
                st_p = misc.tile([P, NKC, NCH, 6], f32, name="st_proj")
                t_ps = []
                for hf in range(2):
                    for mi in range(2):
                        m = 2 * hf + mi
                        pss = [pp_mm.tile([P, 512], f32, name="ps_mm")
                               for _ in range(NCH)]
                        for wi, wt in enumerate((wpT_hi, wpT_lo)):
                            for k in range(NKC):
                                for n_ in range(NCH):
                                    nc.tensor.matmul(
                                        pss[n_], lhsT=wt[:, k, P * m:P * m + P],
                                        rhs=a_spk[:, k, 512 * n_:512 * n_ + 512],
                                        start=(wi == 0 and k == 0),
                                        stop=(wi == 1 and k == NKC - 1))
                        for n_ in range(NCH):
                            nc.any.tensor_copy(y_p[:, m, 512 * n_:512 * n_ + 512],
                                               pss[n_])
                            nc.vector.bn_stats(out=st_p[:, m, n_, :], in_=pss[n_])
                    bout_p = stats_finish(f"proj{hf}", misc,
                                          st_p[:, 2 * hf:2 * hf + 2, :, :], 2)
                    t_ps.append(thresholds(f"proj{hf}", misc, bout_p,
                                           par_proj[:, 2 * hf:2 * hf + 2, :], 0, 2))
                for hf in range(2):
                    for mi in range(2):
                        m = 2 * hf + mi
                        spike(y_p[:, m, :], y_p[:, m, :], t_ps[hf], mi)
                        xc = p_prs.tile([P, COLS], f32, name="xc_res")
                        nc.sync.dma_start(out=xc, in_=part3(x_in[:, :])[:, m, :])
                        xr = p_prs.tile([P, COLS], f32, name="xr_res")
                        nc.gpsimd.tensor_add(xr, y_p[:, m, :], xc)
                        nc.sync.dma_start(out=xres_d[m, :, :], in_=xr)
                        xh = p_prs.tile([P, COLS], bf16, name="xh_res")
                        nc.vector.tensor_copy(xh, xr)
                        nc.scalar.dma_start(out=xrsh_d[m, :, :], in_=xh)
                        xl = p_prs.tile([P, COLS], bf16, name="xl_res")
                        nc.vector.tensor_sub(xl, xr, xh)
                        nc.scalar.dma_start(out=xrsl_d[m, :, :], in_=xl)

        # ========= Phase D: fc1 (fp32) in 8 slices, per-slice AR =========
        with tc.tile_pool(name="p_h1", bufs=1) as p_h1:
            h1 = p_h1.tile([P, NMH, COLS], bf16)

            QM = NMH // 8
            with tc.tile_pool(name="p_f1", bufs=1) as p_f1, \
                 tc.tile_pool(name="p_f1q", bufs=2) as p_f1q:
                # PE warm-keeper: dummy matmuls gated on the proj thresholds so
                # they run during the bridge (spike/residual/split chain) and
                # fc1 starts at full clock instead of HAM-cold.
                warm = p_f1.tile([P, 512], bf16, name="warm")
                nc.vector.tensor_copy(warm[:, 0:2], t_ps[1][:, 0:2])
                wps = pp_mm.tile([P, 512], f32, name="ps_mm")
                for _ in range(40):
                    nc.tensor.matmul(wps, lhsT=warm[:, 0:P], rhs=warm,
                                     start=True, stop=True)
                wf1_hi = p_f1.tile([P, NKC, HID], bf16)
                nc.sync.dma_start(out=wf1_hi, in_=part3(wfc1T_hi_in[:, :]))
                wf1_lo = p_f1.tile([P, NKC, HID], bf16)
                nc.sync.dma_start(out=wf1_lo, in_=part3(wfc1T_lo_in[:, :]))
                xrs_hi = p_f1.tile([P, NKC, COLS], bf16)
                xrs_lo = p_f1.tile([P, NKC, COLS], bf16)
                for k in range(NKC):
                    nc.sync.dma_start(out=xrs_hi[:, k, :], in_=xrsh_d[k, :, :])
                    nc.scalar.dma_start(out=xrs_lo[:, k, :], in_=xrsl_d[k, :, :])
                # 3 passes: hi*hi + hi*lo + lo*hi (lo*lo ~ 2^-18, dropped)
                passes = ((wf1_hi, xrs_hi), (wf1_hi, xrs_lo), (wf1_lo, xrs_hi))
                for qt in range(8):
                    y1q = p_f1q.tile([P, QM, COLS], f32, name="y1q")
                    st_q = misc.tile([P, QM, NCH, 6], f32, name=f"st_fc1q{qt}")
                    for mi in range(QM):
                        m = QM * qt + mi
                        pss = [pp_mm.tile([P, 512], f32, name="ps_mm")
                               for _ in range(NCH)]
                        for pi, (wt, xt) in enumerate(passes):
                            for k in range(NKC):
                                for n_ in range(NCH):
                                    nc.tensor.matmul(
                                        pss[n_], lhsT=wt[:, k, P * m:P * m + P],
                                        rhs=xt[:, k, 512 * n_:512 * n_ + 512],
                                        start=(pi == 0 and k == 0),
                                        stop=(pi == 2 and k == NKC - 1))
                        for n_ in range(NCH):
                            nc.any.tensor_copy(y1q[:, mi, 512 * n_:512 * n_ + 512],
                                               pss[n_])
                            nc.vector.bn_stats(out=st_q[:, mi, n_, :], in_=pss[n_])
                    bout_q = stats_finish(f"fc1q{qt}", misc, st_q, QM)
                    t1q = thresholds(f"fc1q{qt}", misc, bout_q,
                                     par_fc1[:, QM * qt:QM * qt + QM, :], 0, QM)
                    for mi in range(QM):
                        spike(h1[:, QM * qt + mi, :], y1q[:, mi, :], t1q, mi)

            # ========= Phase E: fc2 (split bf16) + residual + out =========
            with tc.tile_pool(name="p_f2", bufs=1) as p_f2, \
                 tc.tile_pool(name="p_f2s", bufs=2) as p_f2s:
                wfc2T_hi = p_f2.tile([P, NMH, C], bf16)
                nc.sync.dma_start(out=wfc2T_hi, in_=part3(wfc2T_hi_in[:, :]))
                wfc2T_lo = p_f2.tile([P, NMH, C], bf16)
                nc.sync.dma_start(out=wfc2T_lo, in_=part3(wfc2T_lo_in[:, :]))

                y2 = p_f2.tile([P, NKC, COLS], f32)
                st2 = misc.tile([P, NKC, NCH, 6], f32, name="st_fc2")
                out3 = part3(out_ext[:, :])
                for m in range(NKC):
                    pss = [pp_mm.tile([P, 512], f32, name="ps_mm")
                           for _ in range(NCH)]
                    for wi, wt in enumerate((wfc2T_hi, wfc2T_lo)):
                        for k in range(NMH):
                            for n_ in range(NCH):
                                nc.tensor.matmul(
                                    pss[n_], lhsT=wt[:, k, P * m:P * m + P],
                                    rhs=h1[:, k, 512 * n_:512 * n_ + 512],
                                    start=(wi == 0 and k == 0),
                                    stop=(wi == 1 and k == NMH - 1))
                    for n_ in range(NCH):
                        nc.any.tensor_copy(y2[:, m, 512 * n_:512 * n_ + 512], pss[n_])
                        nc.vector.bn_stats(out=st2[:, m, n_, :], in_=pss[n_])
                    bout_m = stats_finish(f"fc2m{m}", misc, st2[:, m:m + 1, :, :], 1)
                    t2 = thresholds(f"fc2m{m}", misc, bout_m,
                                    par_fc2[:, m:m + 1, :], 0, 1)
                    spike(y2[:, m, :], y2[:, m, :], t2, 0)
                    xrc = p_f2s.tile([P, COLS], f32, name="xrc")
                    nc.sync.dma_start(out=xrc, in_=xres_d[m, :, :])
                    nc.vector.tensor_add(y2[:, m, :], y2[:, m, :], xrc)
                    nc.sync.dma_start(out=out3[:, m, :], in_=y2[:, m, :])

    nc.compile()
    return nc


def _split_bf16(w):
    import ml_dtypes
    hi = np.ascontiguousarray(w.astype(ml_dtypes.bfloat16))
    lo = np.ascontiguousarray((w - hi.astype(np.float32)).astype(ml_dtypes.bfloat16))
    return hi, lo


def build_inputs(inp):
    """Host-side prep: per-core input maps (weights replicated)."""
    x = inp["x"]

    def thr_pack(g, b, bias):
        A = (2.0 - b) / g
        return np.ascontiguousarray(np.stack([A, bias], axis=1).astype(np.float32))

    wqkvT = np.ascontiguousarray(
        np.concatenate([inp["q_w"].T, inp["k_w"].T, inp["v_w"].T], axis=1))
    wp_hi, wp_lo = _split_bf16(np.ascontiguousarray(inp["proj_w"].T))
    w1_hi, w1_lo = _split_bf16(np.ascontiguousarray(inp["fc1_w"].T))
    w2_hi, w2_lo = _split_bf16(np.ascontiguousarray(inp["fc2_w"].T))

    zc = np.zeros(C, np.float32)
    thr_qkv = np.ascontiguousarray(np.concatenate([
        thr_pack(inp["q_g"], inp["q_b"], zc),
        thr_pack(inp["k_g"], inp["k_b"], zc),
        thr_pack(inp["v_g"], inp["v_b"], zc)], axis=1))

    shared = dict(
        wqkvT=wqkvT, wprojT_hi=wp_hi, wprojT_lo=wp_lo,
        wfc1T_hi=w1_hi, wfc1T_lo=w1_lo,
        wfc2T_hi=w2_hi, wfc2T_lo=w2_lo, thr_qkv=thr_qkv,
        thr_proj=thr_pack(inp["proj_g"], inp["proj_b"], inp["proj_bias"]),
        thr_fc1=thr_pack(inp["fc1_g"], inp["fc1_b"], inp["fc1_bias"]),
        thr_fc2=thr_pack(inp["fc2_g"], inp["fc2_b"], inp["fc2_bias"]))

    in_maps = []
    for i in range(NCORES):
        xl = np.ascontiguousarray(
            np.concatenate([x[BPC * i + b] for b in range(BPC)], axis=1))
        in_maps.append(dict(x=xl, **shared))
    return in_maps


def get_program():
    if "nc" not in _cache:
        _cache["nc"] = _build_program()
    return _cache["nc"]


def run(in_maps, **kwargs):
    _ensure_axon_hooks_shim()
    from concourse.bass_utils import run_bass_kernel_spmd
    nc = get_program()
    return run_bass_kernel_spmd(nc, in_maps, list(range(NCORES)), **kwargs)


def kernel(**inputs):
    inp = {k: np.asarray(v, dtype=np.float32) for k, v in inputs.items()}
    assert inp["x"].shape == (B, C, N), inp["x"].shape
    res = run(build_inputs(inp))
    out = np.empty((B, C, N), np.float32)
    for i in range(NCORES):
        o = res.results[i]["out"]
        for b in range(BPC):
            out[BPC * i + b] = o[:, N * b:N * (b + 1)]
    return out

